# revision 16
# baseline (speedup 1.0000x reference)
"""MeshConv (Chebyshev graph conv, K=6) Trainium2 kernel, 8 NeuronCores.

Strategy: pure batch parallelism (B=8 == n_cores).  Each core owns one batch
and runs the full Chebyshev recursion on its own [M, 64] feature block, so
there are NO collectives at all.  The SpMM uses the TensorEngine: edges are
slotted host-side into per-dst-tile chunks (variable count per tile, sized
to the tile's actual edge load); per chunk a one-hot [128 slots x 128 rows]
pattern (built on device from compact (rloc,val) arrays) is the stationary
operand against 64-wide gathered source rows (f32 gathers: 64 feats * 4B =
256B packets, spread over 4 SWDGE queues).  The dense projection accumulates
k-stripes of transposed activations and finishes with a 3-chunk GEMM against
a k-major-restacked W.  Vertices stay in natural order (no permutation), so
host prep only touches the edge arrays.
"""
import sys

sys.path.insert(0, '/opt/trn_rl_repo')

import numpy as np
import ml_dtypes

import concourse.bass as bass
import concourse.bacc as bacc
import concourse.mybir as mybir
import concourse.tile as tile_mod
from concourse.tile import TileContext
from concourse import bass2jax

# ---------------------------------------------------------------- constants
B, F, K = 8, 64, 6
NCORE = 8
PB = 32          # pattern chunks built per DVE/DMA batch

# walrus in this environment accepts only 1 sync-wait per CTRL instruction:
# spread the Tile tail-drain's waits across preceding nops.
def _patched_drain_and_barrier(self, tick_clock, wait_clock):
    nop0 = self.nc.sync.nop(nofuse=True)
    wait_clock.add_sem_waits(nop0.ins, tile_mod.ScopedClock({None: tick_clock.global_clock}))
    si = nop0.ins.sync_info
    waits = list(si.on_wait) if si and si.on_wait else []
    if len(waits) > 1:
        si.on_wait = waits[:1]
        rest = waits[1:]
        while rest:
            n = self.nc.sync.nop(nofuse=True)
            nsi = n.ins.sync_info
            if nsi is None:
                n.ins.sync_info = mybir.SyncInfo(on_wait=rest[:1], on_update=[])
            else:
                nsi.on_wait = rest[:1]
            rest = rest[1:]
    self.nc.sync.drain()
    self.nc.all_engine_barrier()
    assert self.sems is not None
    popped = self.nc._tile_sem_poison_stack.pop()
    assert popped is self._sem_poison
    self.nc.clear_and_free_semaphores(list(self.sems.allocated().values()))
    self.nc.all_engine_barrier()


tile_mod.TileContext._drain_and_barrier = _patched_drain_and_barrier


class Cfg:
    def __init__(self, M, mpad, asplit, bbase, ga_call, gb_call, G):
        self.M = M
        self.MPAD = mpad
        self.ASPLIT = asplit          # A gathers read rows [0, ASPLIT)
        self.BBASE = bbase            # B gathers read rows [BBASE, MPAD)
        assert asplit <= 32768 and mpad - bbase <= 32768
        self.NT = mpad // 128
        assert mpad % 128 == 0
        self.GA_CALL, self.GB_CALL = ga_call, gb_call
        self.G = G                    # dst tiles per group
        assert self.NT % G == 0
        self.NGRP = self.NT // G


CFG_FULL = Cfg(M=40000, mpad=40960, asplit=32768, bbase=8192,
               ga_call=4096, gb_call=4096, G=4)


def _rup(x, m):
    return (x + m - 1) // m * m


# ---------------------------------------------------------------- host prep
def prep_graph(cfg, edge_rows, edge_cols, edge_vals):
    """Slot the edge list into per-tile variable chunk lists (vectorized).

    Returns wrapped int16 gather indices, compact pattern arrays
    ([128 lanes, NCHV_PAD] f32 rloc/val), and the chunk-grid meta the
    device build needs (per-tile A/B chunk counts and offsets).
    """
    er = np.asarray(edge_rows).astype(np.int64)
    ec = np.asarray(edge_cols).astype(np.int64)
    ev = np.asarray(edge_vals).astype(np.float32)
    E = er.shape[0]

    tile = er >> 7
    cat = np.where(ec >= cfg.ASPLIT, 2, np.where(ec >= cfg.BBASE, 1, 0))
    order = np.argsort((tile << 34) | (cat.astype(np.int64) << 32) | ec, kind="stable")
    tile_s = tile[order]
    ec_s = ec[order]
    ev_s = ev[order]
    rloc_s = er[order] & 127

    n_t = np.bincount(tile_s, minlength=cfg.NT)
    aonly = np.bincount(tile[cat == 0], minlength=cfg.NT)
    bonly = np.bincount(tile[cat == 2], minlength=cfg.NT)
    ct = np.maximum((n_t + 127) >> 7, ((aonly + 127) >> 7) + ((bonly + 127) >> 7))
    cB = (bonly + 127) >> 7
    cA = ct - cB
    if not ((cA * 128 >= aonly).all() and (cA * 128 + cB * 128 >= n_t).all()):
        raise RuntimeError("tile chunk packing infeasible for this edge list")
    nB_t = np.maximum(bonly, n_t - cA * 128)
    nA_t = n_t - nB_t

    chstart = np.zeros(cfg.NT + 1, np.int64)
    np.cumsum(ct, out=chstart[1:])
    baseA = np.zeros(cfg.NT + 1, np.int64)
    np.cumsum(cA, out=baseA[1:])
    baseB = np.zeros(cfg.NT + 1, np.int64)
    np.cumsum(cB, out=baseB[1:])
    NCHV = int(chstart[-1])
    NCHV_PAD = _rup(NCHV, PB)
    NIDXA_PAD = _rup(int(baseA[-1]) * 128, cfg.GA_CALL)
    NIDXB_PAD = _rup(max(int(baseB[-1]), 1) * 128, cfg.GB_CALL)

    cum = np.zeros(cfg.NT + 1, np.int64)
    np.cumsum(n_t, out=cum[1:])
    pos = np.arange(E, dtype=np.int64) - cum[tile_s]
    isA = pos < nA_t[tile_s]

    idxA = np.zeros(NIDXA_PAD, np.int16)
    idxB = np.zeros(NIDXB_PAD, np.int16)
    prloc = np.zeros((128, NCHV_PAD), np.float32)
    pval = np.zeros((128, NCHV_PAD), np.float32)

    sA = pos[isA]
    tA = tile_s[isA]
    laneA = (sA & 127).astype(np.int64)
    jA = sA >> 7
    idxA[(baseA[tA] + jA) * 128 + laneA] = ec_s[isA].astype(np.int16)
    gchA = chstart[tA] + jA
    prloc[laneA, gchA] = rloc_s[isA]
    pval[laneA, gchA] = ev_s[isA]

    nb = ~isA
    sB = (pos - nA_t[tile_s])[nb]
    tB = tile_s[nb]
    laneB = (sB & 127).astype(np.int64)
    jB = sB >> 7
    idxB[(baseB[tB] + jB) * 128 + laneB] = (ec_s[nb] - cfg.BBASE).astype(np.int16)
    gchB = chstart[tB] + cA[tB] + jB
    prloc[laneB, gchB] = rloc_s[nb]
    pval[laneB, gchB] = ev_s[nb]

    meta = {
        "cA": tuple(int(v) for v in cA),
        "cB": tuple(int(v) for v in cB),
        "chstart": tuple(int(v) for v in chstart),
        "baseA": tuple(int(v) for v in baseA),
        "baseB": tuple(int(v) for v in baseB),
        "NCHV_PAD": NCHV_PAD,
        "NIDXA_PAD": NIDXA_PAD,
        "NIDXB_PAD": NIDXB_PAD,
    }
    arrays = {
        "idxA": np.ascontiguousarray(idxA.reshape(-1, 16).T),   # [16, NIDXA_PAD/16]
        "idxB": np.ascontiguousarray(idxB.reshape(-1, 16).T),
        "prloc": prloc,
        "pval": pval,
    }
    return arrays, meta


def prep_w(W):
    """W [F*K, F] (rows fin*K + k) -> k-major stack [K*F, F] (rows k*F + fin)."""
    Wk = np.asarray(W).astype(np.float32).reshape(F, K, F).transpose(1, 0, 2)
    return np.ascontiguousarray(Wk.reshape(K * F, F)).astype(ml_dtypes.bfloat16)


# ---------------------------------------------------------------- device IR
def build_nc(cfg, meta, repeat=1, ablate=(), nq=4):
    nc = bacc.Bacc(None, target_bir_lowering=False, debug=False,
                   dynamic_dma_scratch_size=16384, num_swdge_queues=nq)
    dt = mybir.dt
    G = cfg.G
    aluop = mybir.AluOpType
    cA, cB = meta["cA"], meta["cB"]
    chstart, baseA, baseB = meta["chstart"], meta["baseA"], meta["baseB"]
    NCHV_PAD = meta["NCHV_PAD"]
    NIDXA, NIDXB = meta["NIDXA_PAD"], meta["NIDXB_PAD"]

    xb = nc.declare_dram_parameter("xb", [cfg.M, F], dt.bfloat16, isOutput=False)
    idxA_d = nc.declare_dram_parameter("idxA", [16, NIDXA // 16], dt.int16, isOutput=False)
    idxB_d = nc.declare_dram_parameter("idxB", [16, NIDXB // 16], dt.int16, isOutput=False)
    prloc_d = nc.declare_dram_parameter("prloc", [128, NCHV_PAD], dt.float32, isOutput=False)
    pval_d = nc.declare_dram_parameter("pval", [128, NCHV_PAD], dt.float32, isOutput=False)
    wst_d = nc.declare_dram_parameter("wst", [K * F, F], dt.bfloat16, isOutput=False)
    out_d = nc.declare_dram_parameter("out", [cfg.MPAD, F], dt.bfloat16, isOutput=True)

    xs = [nc.dram_tensor(f"xs{k}", [cfg.MPAD, F], dt.float32) for k in range(K - 1)]
    xT_d = nc.dram_tensor("xT", [K * F, cfg.MPAD], dt.bfloat16)
    patd = nc.dram_tensor("patd", [NCHV_PAD * 128, 128], dt.bfloat16)

    CPG_A = cfg.GA_CALL // 128       # chunks per A gather call
    CPG_B = cfg.GB_CALL // 128
    NCALL_A = NIDXA // cfg.GA_CALL
    NCALL_B = NIDXB // cfg.GB_CALL
    NG0 = cfg.MPAD // (128 * G)      # stage0 groups
    WMAX = max(chstart[g * G + G] - chstart[g * G] for g in range(cfg.NGRP))

    with TileContext(nc) as tc:
        with (
            tc.tile_pool(name="io", bufs=1) as io,
            tc.tile_pool(name="patp", bufs=2) as patp,
            tc.tile_pool(name="ga", bufs=2) as gap,
            tc.tile_pool(name="gb", bufs=2) as gbp,
            tc.tile_pool(name="ev", bufs=2) as evp,
            tc.tile_pool(name="prj", bufs=2) as prjp,
            tc.tile_pool(name="ps", bufs=3, space="PSUM") as psp,
            tc.tile_pool(name="psT", bufs=2, space="PSUM") as psTp,
            tc.tile_pool(name="psg", bufs=2, space="PSUM") as psgp,
        ):
            # ---- resident tiles
            idxA_t = io.tile([128, NIDXA // 16], dt.int16)
            idxB_t = io.tile([128, NIDXB // 16], dt.int16)
            prlocT = io.tile([128, NCHV_PAD], dt.float32)
            pvalT = io.tile([128, NCHV_PAD], dt.float32)
            wsb = io.tile([128, K * F // 128, F], dt.bfloat16)
            iota_i = io.tile([128, 128], dt.int16)
            iota_b = io.tile([128, 128], dt.float32)
            iotaw = io.tile([128, 16, 128], dt.float32)
            pcol_i = io.tile([128, 1], dt.int16)
            pcol_b = io.tile([128, 1], dt.float32)
            ident_t = io.tile([128, 128], dt.bfloat16)

            for i in range(8):
                nc.sync.dma_start(out=idxA_t[16 * i:16 * (i + 1), :], in_=idxA_d[:])
                nc.sync.dma_start(out=idxB_t[16 * i:16 * (i + 1), :], in_=idxB_d[:])
            nc.sync.dma_start(out=prlocT[:], in_=prloc_d[:])
            nc.sync.dma_start(out=pvalT[:], in_=pval_d[:])
            nc.sync.dma_start(out=wsb[:], in_=wst_d[:].rearrange("(j p) f -> p j f", p=128))
            nc.gpsimd.iota(iota_i[:], pattern=[[1, 128]], base=0, channel_multiplier=0)
            nc.vector.tensor_copy(iota_b[:], iota_i[:])
            for _w in range(16):
                nc.vector.tensor_copy(iotaw[:, _w, :], iota_b[:])
            nc.gpsimd.iota(pcol_i[:], pattern=[[0, 1]], base=0, channel_multiplier=1)
            nc.vector.tensor_copy(pcol_b[:], pcol_i[:])
            nc.vector.tensor_scalar(ident_t[:], iota_b[:], pcol_b[:, 0:1], None,
                                    op0=aluop.is_equal)

            patd_v = patd[:].rearrange("(c p) r -> p c r", p=128)

            gshA = gshB = None
            if "gather" in ablate:
                gshA = io.tile([128, CPG_A, F], dt.bfloat16)
                nc.vector.memset(gshA[:], 0.0)
                gshB = io.tile([128, CPG_B, F], dt.bfloat16)
                nc.vector.memset(gshB[:], 0.0)

            def body():
                # ---- pattern build: pat[lane, r] = (r == rloc[lane]) * val[lane]
                for c0 in range(0, NCHV_PAD, PB):
                    pt = patp.tile([128, max(PB, WMAX), 128], dt.bfloat16, tag="pat")
                    for h in range(0, PB, 16):
                        cmp = patp.tile([128, 16, 128], dt.float32, tag="cmp")
                        nc.vector.tensor_tensor(
                            cmp[:], iotaw[:],
                            prlocT[:, c0 + h:c0 + h + 16].broadcast_to((128, 16, 128)),
                            op=aluop.is_equal)
                        nc.vector.tensor_tensor(
                            pt[:, h:h + 16, :], cmp[:],
                            pvalT[:, c0 + h:c0 + h + 16].broadcast_to((128, 16, 128)),
                            op=aluop.mult)
                    nc.sync.dma_start(out=patd_v[:, c0:c0 + PB, :], in_=pt[:, :PB, :])

                # ---- stage0: xb -> xs[0] (f32, zero-padded) + xT stripe 0
                for g in range(NG0):
                    r0 = g * 128 * G
                    nreal = min(max(cfg.M - r0, 0), 128 * G)
                    t0 = evp.tile([128, G, F], dt.bfloat16, tag="t0")
                    if nreal < 128 * G:
                        nc.vector.memset(t0[:], 0.0)
                    ft = nreal // 128
                    if ft:
                        nc.sync.dma_start(
                            out=t0[:, :ft, :],
                            in_=xb[r0:r0 + 128 * ft, :].rearrange("(a p) f -> p a f", p=128))
                    rem = nreal % 128
                    if rem:
                        nc.sync.dma_start(out=t0[:rem, ft, :],
                                          in_=xb[r0 + 128 * ft:r0 + nreal, :])
                    t0f = evp.tile([128, G, F], dt.float32, tag="t0f")
                    nc.vector.tensor_copy(t0f[:], t0[:])
                    nc.sync.dma_start(
                        out=xs[0][r0:r0 + 128 * G, :].rearrange("(a p) f -> p a f", p=128),
                        in_=t0f[:])
                    tp = psTp.tile([64, G, 128], dt.bfloat16, tag="tp")
                    for t in range(G):
                        nc.tensor.transpose(tp[:, t, :], t0[:, t, :], ident_t[:])
                    tps = evp.tile([64, G, 128], dt.bfloat16, tag="tps")
                    nc.vector.tensor_copy(tps[:], tp[:])
                    nc.sync.dma_start(out=xT_d[0:F, r0:r0 + 128 * G], in_=tps[:])

                # ---- Chebyshev steps
                for k in range(1, K):
                    src = xs[k - 1]
                    GAB, GBB = [], []
                    if "gather" in ablate:
                        GAB = [gshA] * NCALL_A
                        GBB = [gshB] * NCALL_B
                    else:
                        for ci in range(NCALL_A):
                            gt = gap.tile([128, CPG_A, F], dt.float32, tag="ga")
                            nc.gpsimd.dma_gather(
                                out_ap=gt[:], in_ap=src[0:cfg.ASPLIT, :],
                                idxs_ap=idxA_t[:, ci * (cfg.GA_CALL // 16):(ci + 1) * (cfg.GA_CALL // 16)],
                                num_idxs=cfg.GA_CALL, num_idxs_reg=cfg.GA_CALL,
                                elem_size=F, single_packet=False, queue_num=ci % nq)
                            gtb = gap.tile([128, CPG_A, F], dt.bfloat16, tag="gab")
                            nc.vector.tensor_copy(gtb[:], gt[:])
                            GAB.append(gtb)
                        for ci in range(NCALL_B):
                            gt = gbp.tile([128, CPG_B, F], dt.float32, tag="gb")
                            nc.gpsimd.dma_gather(
                                out_ap=gt[:], in_ap=src[cfg.BBASE:cfg.MPAD, :],
                                idxs_ap=idxB_t[:, ci * (cfg.GB_CALL // 16):(ci + 1) * (cfg.GB_CALL // 16)],
                                num_idxs=cfg.GB_CALL, num_idxs_reg=cfg.GB_CALL,
                                elem_size=F, single_packet=False, queue_num=(ci + 1) % nq)
                            gtb = gbp.tile([128, CPG_B, F], dt.bfloat16, tag="gbb")
                            nc.vector.tensor_copy(gtb[:], gt[:])
                            GBB.append(gtb)

                    for grp in range(cfg.NGRP):
                        r0 = grp * 128 * G
                        ch0 = chstart[grp * G]
                        w = chstart[grp * G + G] - ch0
                        if w:
                            pt = patp.tile([128, max(PB, WMAX), 128], dt.bfloat16, tag="pat")
                            nc.sync.dma_start(out=pt[:, :w, :],
                                              in_=patd_v[:, ch0:ch0 + w, :])
                        ps = psp.tile([128, G, F], dt.float32, tag="ps")
                        for t in range(G):
                            tid = grp * G + t
                            nch = cA[tid] + cB[tid]
                            if nch == 0:
                                nc.vector.memset(ps[:, t, :], 0.0)
                                continue
                            for j in range(nch):
                                if j < cA[tid]:
                                    ga_i = baseA[tid] + j
                                    mov = GAB[ga_i // CPG_A][:, ga_i % CPG_A, :]
                                else:
                                    gb_i = baseB[tid] + (j - cA[tid])
                                    mov = GBB[gb_i // CPG_B][:, gb_i % CPG_B, :]
                                nc.tensor.matmul(
                                    ps[:, t, :], pt[:, chstart[tid] - ch0 + j, :], mov,
                                    start=(j == 0), stop=(j == nch - 1))
                        xc = evp.tile([128, G, F], dt.float32, tag="xc")
                        nc.sync.dma_start(
                            out=xc[:],
                            in_=src[r0:r0 + 128 * G, :].rearrange("(a p) f -> p a f", p=128))
                        xk_t = evp.tile([128, G, F], dt.float32, tag="xk")
                        if k == 1:
                            nc.vector.tensor_sub(xk_t[:], ps[:], xc[:])
                        else:
                            xp = evp.tile([128, G, F], dt.float32, tag="xp")
                            nc.sync.dma_start(
                                out=xp[:],
                                in_=xs[k - 2][r0:r0 + 128 * G, :].rearrange("(a p) f -> p a f", p=128))
                            tmp = evp.tile([128, G, F], dt.float32, tag="tmp")
                            nc.vector.tensor_sub(tmp[:], ps[:], xc[:])
                            nc.vector.scalar_tensor_tensor(
                                xk_t[:], tmp[:], 2.0, xp[:],
                                op0=aluop.mult, op1=aluop.subtract)
                        if k < K - 1:
                            nc.sync.dma_start(
                                out=xs[k][r0:r0 + 128 * G, :].rearrange("(a p) f -> p a f", p=128),
                                in_=xk_t[:])
                        xkb = evp.tile([128, G, F], dt.bfloat16, tag="xkb")
                        nc.vector.tensor_copy(xkb[:], xk_t[:])
                        tp = psTp.tile([64, G, 128], dt.bfloat16, tag="tp")
                        for t in range(G):
                            nc.tensor.transpose(tp[:, t, :], xkb[:, t, :], ident_t[:])
                        tps = evp.tile([64, G, 128], dt.bfloat16, tag="tps")
                        nc.vector.tensor_copy(tps[:], tp[:])
                        nc.sync.dma_start(out=xT_d[k * F:(k + 1) * F, r0:r0 + 128 * G],
                                          in_=tps[:])

                # ---- dense projection: out = X_cat @ W  (3 stat chunks of 128)
                for grp in range(cfg.NGRP):
                    r0 = grp * 128 * G
                    stx = prjp.tile([128, 3, G * 128], dt.bfloat16, tag="stx")
                    for j in range(3):
                        nc.sync.dma_start(out=stx[:, j, :],
                                          in_=xT_d[128 * j:128 * (j + 1), r0:r0 + 128 * G])
                    pg = psgp.tile([128, G, F], dt.float32, tag="pg")
                    for t in range(G):
                        for j in range(3):
                            nc.tensor.matmul(pg[:, t, :], stx[:, j, t * 128:(t + 1) * 128],
                                             wsb[:, j, :], start=(j == 0), stop=(j == 2))
                    ob = prjp.tile([128, G, F], dt.bfloat16, tag="ob")
                    nc.vector.tensor_copy(ob[:], pg[:])
                    nc.sync.dma_start(
                        out=out_d[r0:r0 + 128 * G, :].rearrange("(a p) f -> p a f", p=128),
                        in_=ob[:])

            for _rep in range(repeat):
                body()

    nc.finalize()
    return nc


_NC_CACHE = {}


def get_nc(cfg, meta, repeat=1, **kw):
    key = (cfg.M, cfg.MPAD, cfg.G, repeat, meta["chstart"], tuple(sorted(kw.items())))
    if key not in _NC_CACHE:
        _NC_CACHE[key] = build_nc(cfg, meta, repeat, **kw)
    return _NC_CACHE[key]


# ---------------------------------------------------------------- entry
def make_in_maps(cfg, x, edge_vals, W, edge_rows, edge_cols):
    arrays, meta = prep_graph(cfg, edge_rows, edge_cols, edge_vals)
    wst = prep_w(W)
    xb16 = np.asarray(x).astype(ml_dtypes.bfloat16)
    in_maps = []
    for c in range(NCORE):
        in_maps.append({
            "xb": xb16[c],
            "idxA": arrays["idxA"],
            "idxB": arrays["idxB"],
            "prloc": arrays["prloc"],
            "pval": arrays["pval"],
            "wst": wst,
        })
    return in_maps, meta


def run(cfg, x, edge_vals, W, edge_rows, edge_cols):
    in_maps, meta = make_in_maps(cfg, x, edge_vals, W, edge_rows, edge_cols)
    nc = get_nc(cfg, meta)
    results = bass2jax.run_bass_via_pjrt(nc, in_maps, n_cores=NCORE)
    Bd = np.asarray(x).shape[0]
    out = np.empty((Bd, cfg.M, F), np.float32)
    for c in range(Bd):
        out[c] = results[c]["out"][:cfg.M].astype(np.float32)
    return out


def kernel(**inputs):
    return run(CFG_FULL, inputs["x"], inputs["edge_vals"], inputs["W"],
               inputs["edge_rows"], inputs["edge_cols"])


# revision 17
# speedup vs baseline: 1.0288x; 1.0288x over previous
"""MeshConv (Chebyshev graph conv, K=6) Trainium2 kernel, 8 NeuronCores.

Strategy: pure batch parallelism (B=8 == n_cores).  Each core owns one batch
and runs the full Chebyshev recursion on its own [M, 64] feature block, so
there are NO collectives at all.  The SpMM uses the TensorEngine: edges are
slotted host-side into per-dst-tile chunks (variable count per tile, sized
to the tile's actual edge load); per chunk a one-hot [128 slots x 128 rows]
pattern (built on device from compact (rloc,val) arrays) is the stationary
operand against 64-wide gathered source rows (f32 gathers: 64 feats * 4B =
256B packets, spread over 4 SWDGE queues).  The dense projection accumulates
k-stripes of transposed activations and finishes with a 3-chunk GEMM against
a k-major-restacked W.  Vertices stay in natural order (no permutation), so
host prep only touches the edge arrays.
"""
import sys

sys.path.insert(0, '/opt/trn_rl_repo')

import numpy as np
import ml_dtypes

import concourse.bass as bass
import concourse.bacc as bacc
import concourse.mybir as mybir
import concourse.tile as tile_mod
from concourse.tile import TileContext
from concourse import bass2jax

# ---------------------------------------------------------------- constants
B, F, K = 8, 64, 6
NCORE = 8
PB = 32          # pattern chunks built per DVE/DMA batch

# walrus in this environment accepts only 1 sync-wait per CTRL instruction:
# spread the Tile tail-drain's waits across preceding nops.
def _patched_drain_and_barrier(self, tick_clock, wait_clock):
    nop0 = self.nc.sync.nop(nofuse=True)
    wait_clock.add_sem_waits(nop0.ins, tile_mod.ScopedClock({None: tick_clock.global_clock}))
    si = nop0.ins.sync_info
    waits = list(si.on_wait) if si and si.on_wait else []
    if len(waits) > 1:
        si.on_wait = waits[:1]
        rest = waits[1:]
        while rest:
            n = self.nc.sync.nop(nofuse=True)
            nsi = n.ins.sync_info
            if nsi is None:
                n.ins.sync_info = mybir.SyncInfo(on_wait=rest[:1], on_update=[])
            else:
                nsi.on_wait = rest[:1]
            rest = rest[1:]
    self.nc.sync.drain()
    self.nc.all_engine_barrier()
    assert self.sems is not None
    popped = self.nc._tile_sem_poison_stack.pop()
    assert popped is self._sem_poison
    self.nc.clear_and_free_semaphores(list(self.sems.allocated().values()))
    self.nc.all_engine_barrier()


tile_mod.TileContext._drain_and_barrier = _patched_drain_and_barrier


class Cfg:
    def __init__(self, M, mpad, asplit, bbase, ga_call, gb_call, G):
        self.M = M
        self.MPAD = mpad
        self.ASPLIT = asplit          # A gathers read rows [0, ASPLIT)
        self.BBASE = bbase            # B gathers read rows [BBASE, MPAD)
        assert asplit <= 32768 and mpad - bbase <= 32768
        self.NT = mpad // 128
        assert mpad % 128 == 0
        self.GA_CALL, self.GB_CALL = ga_call, gb_call
        self.G = G                    # dst tiles per group
        assert self.NT % G == 0
        self.NGRP = self.NT // G


CFG_FULL = Cfg(M=40000, mpad=40960, asplit=32768, bbase=8192,
               ga_call=4096, gb_call=4096, G=4)


def _rup(x, m):
    return (x + m - 1) // m * m


# ---------------------------------------------------------------- host prep
def prep_graph(cfg, edge_rows, edge_cols, edge_vals):
    """Slot the edge list into per-tile variable chunk lists (vectorized).

    Returns wrapped int16 gather indices, compact pattern arrays
    ([128 lanes, NCHV_PAD] f32 rloc/val), and the chunk-grid meta the
    device build needs (per-tile A/B chunk counts and offsets).
    """
    er = np.asarray(edge_rows).astype(np.int64)
    ec = np.asarray(edge_cols).astype(np.int64)
    ev = np.asarray(edge_vals).astype(np.float32)
    E = er.shape[0]

    tile = er >> 7
    cat = np.where(ec >= cfg.ASPLIT, 2, np.where(ec >= cfg.BBASE, 1, 0))
    order = np.argsort((tile << 34) | (cat.astype(np.int64) << 32) | ec, kind="stable")
    tile_s = tile[order]
    ec_s = ec[order]
    ev_s = ev[order]
    rloc_s = er[order] & 127

    n_t = np.bincount(tile_s, minlength=cfg.NT)
    aonly = np.bincount(tile[cat == 0], minlength=cfg.NT)
    bonly = np.bincount(tile[cat == 2], minlength=cfg.NT)
    ct = np.maximum((n_t + 127) >> 7, ((aonly + 127) >> 7) + ((bonly + 127) >> 7))
    cB = (bonly + 127) >> 7
    cA = ct - cB
    if not ((cA * 128 >= aonly).all() and (cA * 128 + cB * 128 >= n_t).all()):
        raise RuntimeError("tile chunk packing infeasible for this edge list")
    nB_t = np.maximum(bonly, n_t - cA * 128)
    nA_t = n_t - nB_t

    chstart = np.zeros(cfg.NT + 1, np.int64)
    np.cumsum(ct, out=chstart[1:])
    baseA = np.zeros(cfg.NT + 1, np.int64)
    np.cumsum(cA, out=baseA[1:])
    baseB = np.zeros(cfg.NT + 1, np.int64)
    np.cumsum(cB, out=baseB[1:])
    NCHV = int(chstart[-1])
    NCHV_PAD = _rup(NCHV, PB)
    NIDXA_PAD = _rup(int(baseA[-1]) * 128, cfg.GA_CALL)
    NIDXB_PAD = _rup(max(int(baseB[-1]), 1) * 128, cfg.GB_CALL)

    cum = np.zeros(cfg.NT + 1, np.int64)
    np.cumsum(n_t, out=cum[1:])
    pos = np.arange(E, dtype=np.int64) - cum[tile_s]
    isA = pos < nA_t[tile_s]

    idxA = np.zeros(NIDXA_PAD, np.int16)
    idxB = np.zeros(NIDXB_PAD, np.int16)
    prloc = np.zeros((128, NCHV_PAD), np.float32)
    pval = np.zeros((128, NCHV_PAD), np.float32)

    sA = pos[isA]
    tA = tile_s[isA]
    laneA = (sA & 127).astype(np.int64)
    jA = sA >> 7
    idxA[(baseA[tA] + jA) * 128 + laneA] = ec_s[isA].astype(np.int16)
    gchA = chstart[tA] + jA
    prloc[laneA, gchA] = rloc_s[isA]
    pval[laneA, gchA] = ev_s[isA]

    nb = ~isA
    sB = (pos - nA_t[tile_s])[nb]
    tB = tile_s[nb]
    laneB = (sB & 127).astype(np.int64)
    jB = sB >> 7
    idxB[(baseB[tB] + jB) * 128 + laneB] = (ec_s[nb] - cfg.BBASE).astype(np.int16)
    gchB = chstart[tB] + cA[tB] + jB
    prloc[laneB, gchB] = rloc_s[nb]
    pval[laneB, gchB] = ev_s[nb]

    meta = {
        "cA": tuple(int(v) for v in cA),
        "cB": tuple(int(v) for v in cB),
        "chstart": tuple(int(v) for v in chstart),
        "baseA": tuple(int(v) for v in baseA),
        "baseB": tuple(int(v) for v in baseB),
        "NCHV_PAD": NCHV_PAD,
        "NIDXA_PAD": NIDXA_PAD,
        "NIDXB_PAD": NIDXB_PAD,
    }
    arrays = {
        "idxA": np.ascontiguousarray(idxA.reshape(-1, 16).T),   # [16, NIDXA_PAD/16]
        "idxB": np.ascontiguousarray(idxB.reshape(-1, 16).T),
        "prloc": prloc,
        "pval": pval,
    }
    return arrays, meta


def prep_w(W):
    """W [F*K, F] (rows fin*K + k) -> k-major stack [K*F, F] (rows k*F + fin)."""
    Wk = np.asarray(W).astype(np.float32).reshape(F, K, F).transpose(1, 0, 2)
    return np.ascontiguousarray(Wk.reshape(K * F, F)).astype(ml_dtypes.bfloat16)


# ---------------------------------------------------------------- device IR
def build_nc(cfg, meta, repeat=1, ablate=(), nq=4):
    nc = bacc.Bacc(None, target_bir_lowering=False, debug=False,
                   dynamic_dma_scratch_size=16384, num_swdge_queues=nq)
    dt = mybir.dt
    G = cfg.G
    aluop = mybir.AluOpType
    cA, cB = meta["cA"], meta["cB"]
    chstart, baseA, baseB = meta["chstart"], meta["baseA"], meta["baseB"]
    NCHV_PAD = meta["NCHV_PAD"]
    NIDXA, NIDXB = meta["NIDXA_PAD"], meta["NIDXB_PAD"]

    xb = nc.declare_dram_parameter("xb", [cfg.M, F], dt.bfloat16, isOutput=False)
    idxA_d = nc.declare_dram_parameter("idxA", [16, NIDXA // 16], dt.int16, isOutput=False)
    idxB_d = nc.declare_dram_parameter("idxB", [16, NIDXB // 16], dt.int16, isOutput=False)
    prloc_d = nc.declare_dram_parameter("prloc", [128, NCHV_PAD], dt.float32, isOutput=False)
    pval_d = nc.declare_dram_parameter("pval", [128, NCHV_PAD], dt.float32, isOutput=False)
    wst_d = nc.declare_dram_parameter("wst", [K * F, F], dt.bfloat16, isOutput=False)
    out_d = nc.declare_dram_parameter("out", [cfg.MPAD, F], dt.bfloat16, isOutput=True)

    xs = [nc.dram_tensor(f"xs{k}", [cfg.MPAD, F], dt.float32) for k in range(K - 1)]
    xT_d = nc.dram_tensor("xT", [K * F, cfg.MPAD], dt.bfloat16)
    patd = nc.dram_tensor("patd", [NCHV_PAD * 128, 128], dt.bfloat16)

    CPG_A = cfg.GA_CALL // 128       # chunks per A gather call
    CPG_B = cfg.GB_CALL // 128
    NCALL_A = NIDXA // cfg.GA_CALL
    NCALL_B = NIDXB // cfg.GB_CALL
    NG0 = cfg.MPAD // (128 * G)      # stage0 groups
    WMAX = max(chstart[g * G + G] - chstart[g * G] for g in range(cfg.NGRP))

    with TileContext(nc) as tc:
        with (
            tc.tile_pool(name="io", bufs=1) as io,
            tc.tile_pool(name="patp", bufs=2) as patp,
            tc.tile_pool(name="ga", bufs=2) as gap,
            tc.tile_pool(name="gb", bufs=2) as gbp,
            tc.tile_pool(name="ev", bufs=2) as evp,
            tc.tile_pool(name="prj", bufs=2) as prjp,
            tc.tile_pool(name="ps", bufs=3, space="PSUM") as psp,
            tc.tile_pool(name="psT", bufs=2, space="PSUM") as psTp,
            tc.tile_pool(name="psg", bufs=2, space="PSUM") as psgp,
        ):
            # ---- resident tiles
            idxA_t = io.tile([128, NIDXA // 16], dt.int16)
            idxB_t = io.tile([128, NIDXB // 16], dt.int16)
            prlocT = io.tile([128, NCHV_PAD], dt.float32)
            pvalT = io.tile([128, NCHV_PAD], dt.float32)
            wsb = io.tile([128, K * F // 128, F], dt.bfloat16)
            iota_i = io.tile([128, 128], dt.int16)
            iota_b = io.tile([128, 128], dt.float32)
            pcol_i = io.tile([128, 1], dt.int16)
            pcol_b = io.tile([128, 1], dt.float32)
            ident_t = io.tile([128, 128], dt.bfloat16)

            for i in range(8):
                nc.sync.dma_start(out=idxA_t[16 * i:16 * (i + 1), :], in_=idxA_d[:])
                nc.sync.dma_start(out=idxB_t[16 * i:16 * (i + 1), :], in_=idxB_d[:])
            nc.sync.dma_start(out=prlocT[:], in_=prloc_d[:])
            nc.sync.dma_start(out=pvalT[:], in_=pval_d[:])
            nc.sync.dma_start(out=wsb[:], in_=wst_d[:].rearrange("(j p) f -> p j f", p=128))
            nc.gpsimd.iota(iota_i[:], pattern=[[1, 128]], base=0, channel_multiplier=0)
            nc.vector.tensor_copy(iota_b[:], iota_i[:])
            nc.gpsimd.iota(pcol_i[:], pattern=[[0, 1]], base=0, channel_multiplier=1)
            nc.vector.tensor_copy(pcol_b[:], pcol_i[:])
            nc.vector.tensor_scalar(ident_t[:], iota_b[:], pcol_b[:, 0:1], None,
                                    op0=aluop.is_equal)

            patd_v = patd[:].rearrange("(c p) r -> p c r", p=128)

            gshA = gshB = None
            if "gather" in ablate:
                gshA = io.tile([128, CPG_A, F], dt.bfloat16)
                nc.vector.memset(gshA[:], 0.0)
                gshB = io.tile([128, CPG_B, F], dt.bfloat16)
                nc.vector.memset(gshB[:], 0.0)

            def body():
                # ---- pattern build: pat[lane, r] = (r == rloc[lane]) * val[lane]
                for c0 in range(0, NCHV_PAD, PB):
                    pt = patp.tile([128, max(PB, WMAX), 128], dt.bfloat16, tag="pat")
                    for j in range(PB):
                        nc.vector.tensor_scalar(
                            pt[:, j, :], iota_b[:], prlocT[:, c0 + j:c0 + j + 1],
                            pvalT[:, c0 + j:c0 + j + 1],
                            op0=aluop.is_equal, op1=aluop.mult)
                    nc.sync.dma_start(out=patd_v[:, c0:c0 + PB, :], in_=pt[:, :PB, :])

                # ---- stage0: xb -> xs[0] (f32, zero-padded) + xT stripe 0
                for g in range(NG0):
                    r0 = g * 128 * G
                    nreal = min(max(cfg.M - r0, 0), 128 * G)
                    t0 = evp.tile([128, G, F], dt.bfloat16, tag="t0")
                    if nreal < 128 * G:
                        nc.vector.memset(t0[:], 0.0)
                    ft = nreal // 128
                    if ft:
                        nc.sync.dma_start(
                            out=t0[:, :ft, :],
                            in_=xb[r0:r0 + 128 * ft, :].rearrange("(a p) f -> p a f", p=128))
                    rem = nreal % 128
                    if rem:
                        nc.sync.dma_start(out=t0[:rem, ft, :],
                                          in_=xb[r0 + 128 * ft:r0 + nreal, :])
                    t0f = evp.tile([128, G, F], dt.float32, tag="t0f")
                    nc.vector.tensor_copy(t0f[:], t0[:])
                    nc.sync.dma_start(
                        out=xs[0][r0:r0 + 128 * G, :].rearrange("(a p) f -> p a f", p=128),
                        in_=t0f[:])
                    tp = psTp.tile([64, G, 128], dt.bfloat16, tag="tp")
                    for t in range(G):
                        nc.tensor.transpose(tp[:, t, :], t0[:, t, :], ident_t[:])
                    tps = evp.tile([64, G, 128], dt.bfloat16, tag="tps")
                    nc.vector.tensor_copy(tps[:], tp[:])
                    nc.sync.dma_start(out=xT_d[0:F, r0:r0 + 128 * G], in_=tps[:])

                # ---- Chebyshev steps
                for k in range(1, K):
                    src = xs[k - 1]
                    GAB, GBB = [], []
                    if "gather" in ablate:
                        GAB = [gshA] * NCALL_A
                        GBB = [gshB] * NCALL_B
                    else:
                        for ci in range(NCALL_A):
                            gt = gap.tile([128, CPG_A, F], dt.float32, tag="ga")
                            nc.gpsimd.dma_gather(
                                out_ap=gt[:], in_ap=src[0:cfg.ASPLIT, :],
                                idxs_ap=idxA_t[:, ci * (cfg.GA_CALL // 16):(ci + 1) * (cfg.GA_CALL // 16)],
                                num_idxs=cfg.GA_CALL, num_idxs_reg=cfg.GA_CALL,
                                elem_size=F, single_packet=False, queue_num=ci % nq)
                            gtb = gap.tile([128, CPG_A, F], dt.bfloat16, tag="gab")
                            nc.vector.tensor_copy(gtb[:], gt[:])
                            GAB.append(gtb)
                        for ci in range(NCALL_B):
                            gt = gbp.tile([128, CPG_B, F], dt.float32, tag="gb")
                            nc.gpsimd.dma_gather(
                                out_ap=gt[:], in_ap=src[cfg.BBASE:cfg.MPAD, :],
                                idxs_ap=idxB_t[:, ci * (cfg.GB_CALL // 16):(ci + 1) * (cfg.GB_CALL // 16)],
                                num_idxs=cfg.GB_CALL, num_idxs_reg=cfg.GB_CALL,
                                elem_size=F, single_packet=False, queue_num=(ci + 1) % nq)
                            gtb = gbp.tile([128, CPG_B, F], dt.bfloat16, tag="gbb")
                            nc.vector.tensor_copy(gtb[:], gt[:])
                            GBB.append(gtb)

                    for grp in range(cfg.NGRP):
                        r0 = grp * 128 * G
                        ch0 = chstart[grp * G]
                        w = chstart[grp * G + G] - ch0
                        if w:
                            pt = patp.tile([128, max(PB, WMAX), 128], dt.bfloat16, tag="pat")
                            nc.sync.dma_start(out=pt[:, :w, :],
                                              in_=patd_v[:, ch0:ch0 + w, :])
                        ps = psp.tile([128, G, F], dt.float32, tag="ps")
                        for t in range(G):
                            tid = grp * G + t
                            nch = cA[tid] + cB[tid]
                            if nch == 0:
                                nc.vector.memset(ps[:, t, :], 0.0)
                                continue
                            for j in range(nch):
                                if j < cA[tid]:
                                    ga_i = baseA[tid] + j
                                    mov = GAB[ga_i // CPG_A][:, ga_i % CPG_A, :]
                                else:
                                    gb_i = baseB[tid] + (j - cA[tid])
                                    mov = GBB[gb_i // CPG_B][:, gb_i % CPG_B, :]
                                nc.tensor.matmul(
                                    ps[:, t, :], pt[:, chstart[tid] - ch0 + j, :], mov,
                                    start=(j == 0), stop=(j == nch - 1))
                        xc = evp.tile([128, G, F], dt.float32, tag="xc")
                        nc.sync.dma_start(
                            out=xc[:],
                            in_=src[r0:r0 + 128 * G, :].rearrange("(a p) f -> p a f", p=128))
                        xk_t = evp.tile([128, G, F], dt.float32, tag="xk")
                        if k == 1:
                            nc.vector.tensor_sub(xk_t[:], ps[:], xc[:])
                        else:
                            xp = evp.tile([128, G, F], dt.float32, tag="xp")
                            nc.sync.dma_start(
                                out=xp[:],
                                in_=xs[k - 2][r0:r0 + 128 * G, :].rearrange("(a p) f -> p a f", p=128))
                            tmp = evp.tile([128, G, F], dt.float32, tag="tmp")
                            nc.vector.tensor_sub(tmp[:], ps[:], xc[:])
                            nc.vector.scalar_tensor_tensor(
                                xk_t[:], tmp[:], 2.0, xp[:],
                                op0=aluop.mult, op1=aluop.subtract)
                        if k < K - 1:
                            nc.sync.dma_start(
                                out=xs[k][r0:r0 + 128 * G, :].rearrange("(a p) f -> p a f", p=128),
                                in_=xk_t[:])
                        xkb = evp.tile([128, G, F], dt.bfloat16, tag="xkb")
                        nc.vector.tensor_copy(xkb[:], xk_t[:])
                        tp = psTp.tile([64, G, 128], dt.bfloat16, tag="tp")
                        for t in range(G):
                            nc.tensor.transpose(tp[:, t, :], xkb[:, t, :], ident_t[:])
                        tps = evp.tile([64, G, 128], dt.bfloat16, tag="tps")
                        nc.vector.tensor_copy(tps[:], tp[:])
                        nc.sync.dma_start(out=xT_d[k * F:(k + 1) * F, r0:r0 + 128 * G],
                                          in_=tps[:])

                # ---- dense projection: out = X_cat @ W  (3 stat chunks of 128)
                for grp in range(cfg.NGRP):
                    r0 = grp * 128 * G
                    stx = prjp.tile([128, 3, G * 128], dt.bfloat16, tag="stx")
                    for j in range(3):
                        nc.sync.dma_start(out=stx[:, j, :],
                                          in_=xT_d[128 * j:128 * (j + 1), r0:r0 + 128 * G])
                    pg = psgp.tile([128, G, F], dt.float32, tag="pg")
                    for t in range(G):
                        for j in range(3):
                            nc.tensor.matmul(pg[:, t, :], stx[:, j, t * 128:(t + 1) * 128],
                                             wsb[:, j, :], start=(j == 0), stop=(j == 2))
                    ob = prjp.tile([128, G, F], dt.bfloat16, tag="ob")
                    nc.vector.tensor_copy(ob[:], pg[:])
                    nc.sync.dma_start(
                        out=out_d[r0:r0 + 128 * G, :].rearrange("(a p) f -> p a f", p=128),
                        in_=ob[:])

            for _rep in range(repeat):
                body()

    nc.finalize()
    return nc


_NC_CACHE = {}


def get_nc(cfg, meta, repeat=1, **kw):
    key = (cfg.M, cfg.MPAD, cfg.G, repeat, meta["chstart"], tuple(sorted(kw.items())))
    if key not in _NC_CACHE:
        _NC_CACHE[key] = build_nc(cfg, meta, repeat, **kw)
    return _NC_CACHE[key]


# ---------------------------------------------------------------- entry
def make_in_maps(cfg, x, edge_vals, W, edge_rows, edge_cols):
    arrays, meta = prep_graph(cfg, edge_rows, edge_cols, edge_vals)
    wst = prep_w(W)
    xb16 = np.asarray(x).astype(ml_dtypes.bfloat16)
    in_maps = []
    for c in range(NCORE):
        in_maps.append({
            "xb": xb16[c],
            "idxA": arrays["idxA"],
            "idxB": arrays["idxB"],
            "prloc": arrays["prloc"],
            "pval": arrays["pval"],
            "wst": wst,
        })
    return in_maps, meta


def run(cfg, x, edge_vals, W, edge_rows, edge_cols):
    in_maps, meta = make_in_maps(cfg, x, edge_vals, W, edge_rows, edge_cols)
    nc = get_nc(cfg, meta)
    results = bass2jax.run_bass_via_pjrt(nc, in_maps, n_cores=NCORE)
    Bd = np.asarray(x).shape[0]
    out = np.empty((Bd, cfg.M, F), np.float32)
    for c in range(Bd):
        out[c] = results[c]["out"][:cfg.M].astype(np.float32)
    return out


def kernel(**inputs):
    return run(CFG_FULL, inputs["x"], inputs["edge_vals"], inputs["W"],
               inputs["edge_rows"], inputs["edge_cols"])
